# revision 9
# baseline (speedup 1.0000x reference)
"""Trainium2 Bass kernel: cross-attention transformer block (sigmoid attention).

Sharding (8 NeuronCores): data-parallel over batch B=4 (pairs of cores),
tensor-parallel degree 2 within each pair (8 of 16 heads per core for
attention; 512 of 1024 query tokens per core for merge/FFN; one pair-wise
ReduceScatter over token halves re-shards between the two).

Matmuls run in bf16 (fp32 accumulate in PSUM).  The att_map output is
produced transposed ([kk, q]) so that the att @ v matmul needs no on-chip
transpose; the host assembles the final output as a transposed view.

SBUF tags are shared between phase-disjoint tensors of equal slot size to
keep the static footprint under the 192 KB/partition budget.
"""

import sys

if "/opt/trn_rl_repo" not in sys.path:
    sys.path.insert(0, "/opt/trn_rl_repo")

import ml_dtypes
import numpy as np

import concourse.bass as bass
import concourse.tile as tile
from concourse import bacc, mybir
from concourse.bass_utils import run_bass_kernel_spmd
from concourse.masks import make_identity

BF16 = mybir.dt.bfloat16
F32 = mybir.dt.float32
AF = mybir.ActivationFunctionType
OP = mybir.AluOpType

B, LQ, LK, H, NH, DH, FF = 4, 1024, 2048, 1024, 16, 64, 2048
EPS = 1e-6
NCORES = 8
HL = H // 2     # local head channels (8 heads x 64)
TL = LQ // 2    # local tokens for merge/FFN
NHL = NH // 2   # local heads
P = 128

LAST_EXEC_NS = None
LAST_RESULT = None
_BUILT = None


def _ln(nc, mp, hin, out_ap, g_b, be_b):
    """out = g * (hin - mean) / (std_ddof1 + eps) + be, over free dim (H)."""
    stats = mp.tile([P, 2, 6], F32, tag="t_stats", bufs=2)
    nc.vector.bn_stats(out=stats[:, 0, :], in_=hin[:, 0:512])
    nc.vector.bn_stats(out=stats[:, 1, :], in_=hin[:, 512:1024])
    mv = mp.tile([P, 2], F32, tag="t_mv", bufs=2)
    nc.vector.bn_aggr(out=mv, in_=stats)
    std = mp.tile([P, 1], F32, tag="t_std", bufs=2)
    nc.scalar.activation(out=std, in_=mv[:, 1:2], func=AF.Sqrt,
                         scale=float(H) / float(H - 1))
    nc.vector.tensor_scalar_add(out=std, in0=std, scalar1=EPS)
    rstd = mp.tile([P, 1], F32, tag="t_rstd", bufs=2)
    nc.vector.reciprocal(out=rstd, in_=std)
    nc.vector.tensor_scalar(out=out_ap, in0=hin, scalar1=mv[:, 0:1],
                            scalar2=rstd, op0=OP.subtract, op1=OP.mult)
    nc.vector.tensor_mul(out=out_ap, in0=out_ap, in1=g_b)
    nc.vector.tensor_add(out=out_ap, in0=out_ap, in1=be_b)


def _build():
    nc = bacc.Bacc("TRN2", target_bir_lowering=False, debug=False,
                   num_devices=NCORES)

    # ---- DRAM I/O (per core) ----
    xT_in = nc.dram_tensor("xT", [H, LQ], BF16, kind="ExternalInput")
    yT_in = nc.dram_tensor("yT", [H, LK], BF16, kind="ExternalInput")
    xres_in = nc.dram_tensor("xres", [TL, H], F32, kind="ExternalInput")
    wq_in = nc.dram_tensor("wq", [H, HL], BF16, kind="ExternalInput")
    wk_in = nc.dram_tensor("wk", [H, HL], BF16, kind="ExternalInput")
    wv_in = nc.dram_tensor("wv", [H, HL], BF16, kind="ExternalInput")
    wm_in = nc.dram_tensor("wm", [HL, H], BF16, kind="ExternalInput")
    w1_in = nc.dram_tensor("w1", [H, FF], BF16, kind="ExternalInput")
    w2_in = nc.dram_tensor("w2", [FF, H], BF16, kind="ExternalInput")
    bq_in = nc.dram_tensor("bq", [HL], F32, kind="ExternalInput")
    bk_in = nc.dram_tensor("bk", [HL], F32, kind="ExternalInput")
    bv_in = nc.dram_tensor("bv", [HL], BF16, kind="ExternalInput")
    b1_in = nc.dram_tensor("b1", [FF], F32, kind="ExternalInput")
    b2_in = nc.dram_tensor("b2", [H], BF16, kind="ExternalInput")
    g1_in = nc.dram_tensor("g1", [H], BF16, kind="ExternalInput")
    be1_in = nc.dram_tensor("be1", [H], BF16, kind="ExternalInput")
    g2_in = nc.dram_tensor("g2", [H], BF16, kind="ExternalInput")
    be2_in = nc.dram_tensor("be2", [H], BF16, kind="ExternalInput")

    att_out = nc.dram_tensor("att_out", [NHL, LK, LQ], BF16,
                             kind="ExternalOutput")
    out_tok = nc.dram_tensor("out_tok", [TL, H], F32, kind="ExternalOutput")

    KC = H // P       # 8 contraction tiles over H
    NQ = LQ // 512    # 2

    with tile.TileContext(nc) as tc, \
         tc.tile_pool(name="main", bufs=1) as mp, \
         tc.tile_pool(name="psum", bufs=2, space="PSUM") as pp, \
         tc.tile_pool(name="dram", bufs=1, space="DRAM") as dp:

        # ---- constants ----
        ident = mp.tile([P, P], F32, tag="ident")
        make_identity(nc, ident)
        bq_t = mp.tile([P, HL // P], F32, tag="bq_t")
        nc.gpsimd.dma_start(out=bq_t, in_=bq_in.ap().rearrange("(n p) -> p n", p=P))
        bk_t = mp.tile([P, HL // P], F32, tag="bk_t")
        nc.gpsimd.dma_start(out=bk_t, in_=bk_in.ap().rearrange("(n p) -> p n", p=P))
        b1_t = mp.tile([P, FF // P], F32, tag="b1_t")
        nc.gpsimd.dma_start(out=b1_t, in_=b1_in.ap().rearrange("(n p) -> p n", p=P))
        bv_b = mp.tile([P, HL], BF16, tag="bv_b")
        nc.gpsimd.dma_start(out=bv_b, in_=bv_in.ap().partition_broadcast(P))
        b2_b = mp.tile([P, H], BF16, tag="b2_b")
        nc.gpsimd.dma_start(out=b2_b, in_=b2_in.ap().partition_broadcast(P))
        g1_b = mp.tile([P, H], BF16, tag="g1_b")
        nc.gpsimd.dma_start(out=g1_b, in_=g1_in.ap().partition_broadcast(P))
        be1_b = mp.tile([P, H], BF16, tag="be1_b")
        nc.gpsimd.dma_start(out=be1_b, in_=be1_in.ap().partition_broadcast(P))
        g2_b = mp.tile([P, H], BF16, tag="g2_b")
        nc.gpsimd.dma_start(out=g2_b, in_=g2_in.ap().partition_broadcast(P))
        be2_b = mp.tile([P, H], BF16, tag="be2_b")
        nc.gpsimd.dma_start(out=be2_b, in_=be2_in.ap().partition_broadcast(P))

        # ---- phase A inputs (slots reused later; see tag pairs) ----
        # DMA queues: sync (xT/wq/wk first: they gate the first matmuls),
        # scalar (yT/wv), gpsimd (the rest) - three parallel paths.
        xT = mp.tile([P, KC, LQ], BF16, tag="t_xT")          # 16K -> mergeA
        nc.sync.dma_start(out=xT, in_=xT_in.ap().rearrange("(n p) t -> p n t", p=P))
        wq_s = mp.tile([P, KC, HL], BF16, tag="t_8a")        # 8K -> attedT
        nc.sync.dma_start(out=wq_s, in_=wq_in.ap().rearrange("(n p) f -> p n f", p=P))
        wk_s = mp.tile([P, KC, HL], BF16, tag="t_8b")        # 8K -> h1T
        nc.sync.dma_start(out=wk_s, in_=wk_in.ap().rearrange("(n p) f -> p n f", p=P))
        yT = mp.tile([P, KC, LK], BF16, tag="t_yT")          # 32K -> w1
        nc.scalar.dma_start(out=yT, in_=yT_in.ap().rearrange("(n p) t -> p n t", p=P))
        wv_s = mp.tile([P, KC, HL], BF16, tag="t_8c")        # 8K -> merge_redA
        nc.scalar.dma_start(out=wv_s, in_=wv_in.ap().rearrange("(n p) f -> p n f", p=P))
        wm_s = mp.tile([P, HL // P, H], BF16, tag="t_16a")   # 8K -> midT
        nc.gpsimd.dma_start(out=wm_s, in_=wm_in.ap().rearrange("(n p) f -> p n f", p=P))
        w1_s = mp.tile([P, KC, FF], BF16, tag="t_yT")        # 32K (after yT)
        nc.gpsimd.dma_start(out=w1_s, in_=w1_in.ap().rearrange("(n p) f -> p n f", p=P))

        qT = mp.tile([P, HL // P, LQ], BF16, tag="t_qT")     # 8K -> merge_redB
        kT = mp.tile([P, HL // P, LK], BF16, tag="t_16b")    # 16K -> mergeB -> xres
        vv = mp.tile([P, LK // P, HL], BF16, tag="t_16c")    # 16K -> w2a

        # q projection -> qT [hd, tok]
        for mt in range(HL // P):
            for nt in range(NQ):
                ps = pp.tile([P, 512], F32, tag="ps_small")
                for kt in range(KC):
                    nc.tensor.matmul(ps, wq_s[:, kt, mt * P:(mt + 1) * P],
                                     xT[:, kt, nt * 512:(nt + 1) * 512],
                                     start=(kt == 0), stop=(kt == KC - 1))
                nc.vector.tensor_scalar_add(
                    out=qT[:, mt, nt * 512:(nt + 1) * 512], in0=ps,
                    scalar1=bq_t[:, mt:mt + 1])

        # v projection -> vv [tok, hd]
        for tt in range(LK // P):
            ps = pp.tile([P, 512], F32, tag="ps_small")
            for kt in range(KC):
                nc.tensor.matmul(ps, yT[:, kt, tt * P:(tt + 1) * P],
                                 wv_s[:, kt, :],
                                 start=(kt == 0), stop=(kt == KC - 1))
            nc.vector.tensor_add(out=vv[:, tt, :], in0=ps, in1=bv_b)

        # ---- attention (head pairs), k-projection interleaved per pair ----
        attedT = mp.tile([P, HL // P, LQ], BF16, tag="t_8a")  # 8K (wq slot)
        att_r = att_out.ap().rearrange("h (n p) q -> h p n q", p=P)
        rg = [[0, 1], [2, 3], [4, 5], [6, 7]]

        for pair in range(NHL // 2):
            hc = pair
            # k projection for this pair's chunk -> kT[:, hc, :]
            for nt in range(LK // 512):
                ps = pp.tile([P, 512], F32, tag="ps_small")
                for kt in range(KC):
                    nc.tensor.matmul(ps, wk_s[:, kt, hc * P:(hc + 1) * P],
                                     yT[:, kt, nt * 512:(nt + 1) * 512],
                                     start=(kt == 0), stop=(kt == KC - 1))
                nc.vector.tensor_scalar_add(
                    out=kT[:, hc, nt * 512:(nt + 1) * 512], in0=ps,
                    scalar1=bk_t[:, hc:hc + 1])

            psa = [pp.tile([P, 512], F32, tag="ps_att", bufs=2,
                           name=f"psa{pair}_{qn}") for qn in range(NQ)]
            for hp_i in range(2):
                h = 2 * pair + hp_i
                hp = hp_i * 64
                for c2 in range(2):
                    attT = mp.tile([P, 8, LQ], BF16, tag="t_attT", bufs=2,
                                   name=f"attT{h}_{c2}")
                    for kx in range(8):
                        kkt = c2 * 8 + kx
                        pss = pp.tile([P, LQ], F32, tag="ps_big")
                        for qn in range(NQ):
                            nc.tensor.matmul(
                                pss[:, qn * 512:(qn + 1) * 512],
                                kT[hp:hp + 64, hc, kkt * P:(kkt + 1) * P],
                                qT[hp:hp + 64, hc, qn * 512:(qn + 1) * 512],
                                start=True, stop=True)
                        nc.scalar.activation(out=attT[:, kx, :], in_=pss,
                                             func=AF.Sigmoid, scale=1.0 / 8.0)
                    nc.sync.dma_start(out=att_r[h, :, c2 * 8:(c2 + 1) * 8, :],
                                      in_=attT)
                    for kx in range(8):
                        kkt = c2 * 8 + kx
                        for qn in range(NQ):
                            nc.tensor.matmul(
                                psa[qn][hp:hp + 64, :],
                                vv[:, kkt, h * DH:(h + 1) * DH],
                                attT[:, kx, qn * 512:(qn + 1) * 512],
                                start=(kkt == 0), stop=(kkt == LK // P - 1),
                                tile_position=(0, hp))
            for qn in range(NQ):
                nc.vector.tensor_copy(
                    out=attedT[:, hc, qn * 512:(qn + 1) * 512], in_=psa[qn])

            # After pairs 0,1: merge partial A (head-pairs 0-1) + RS_A so the
            # first collective overlaps the second half of attention.
            if pair == 1:
                merge_sbA = mp.tile([P, LQ // P, H], BF16, tag="t_xT")
                for mt in range(LQ // P):
                    psm = pp.tile([P, H], F32, tag="ps_big")
                    for nt in range(H // 512):
                        for kt in (0, 1):
                            nc.tensor.matmul(
                                psm[:, nt * 512:(nt + 1) * 512],
                                attedT[:, kt, mt * P:(mt + 1) * P],
                                wm_s[:, kt, nt * 512:(nt + 1) * 512],
                                start=(kt == 0), stop=(kt == 1))
                    nc.vector.tensor_copy(out=merge_sbA[:, mt, :], in_=psm)
                rs_inA = dp.tile([LQ, H], BF16)
                rs_outA = dp.tile([TL, H], BF16)
                nc.gpsimd.dma_start(
                    out=rs_inA.rearrange("(n p) c -> p n c", p=P),
                    in_=merge_sbA[:])
                nc.gpsimd.collective_compute(
                    "ReduceScatter", OP.add, replica_groups=rg,
                    ins=[rs_inA.opt()], outs=[rs_outA.opt()])
                merge_redA = mp.tile([P, TL // P, H], BF16, tag="t_8c")
                nc.gpsimd.dma_start(
                    out=merge_redA,
                    in_=rs_outA.rearrange("(n p) c -> p n c", p=P))

        # ---- merge partial B (head-pairs 2-3) + RS_B ----
        merge_sbB = mp.tile([P, LQ // P, H], BF16, tag="t_16b")
        for mt in range(LQ // P):
            psm = pp.tile([P, H], F32, tag="ps_big")
            for nt in range(H // 512):
                for kt in (2, 3):
                    nc.tensor.matmul(
                        psm[:, nt * 512:(nt + 1) * 512],
                        attedT[:, kt, mt * P:(mt + 1) * P],
                        wm_s[:, kt, nt * 512:(nt + 1) * 512],
                        start=(kt == 2), stop=(kt == 3))
            nc.vector.tensor_copy(out=merge_sbB[:, mt, :], in_=psm)
        rs_inB = dp.tile([LQ, H], BF16)
        rs_outB = dp.tile([TL, H], BF16)
        nc.gpsimd.dma_start(
            out=rs_inB.rearrange("(n p) c -> p n c", p=P), in_=merge_sbB[:])
        nc.gpsimd.collective_compute(
            "ReduceScatter", OP.add, replica_groups=rg,
            ins=[rs_inB.opt()], outs=[rs_outB.opt()])
        merge_redB = mp.tile([P, TL // P, H], BF16, tag="t_qT")
        nc.gpsimd.dma_start(out=merge_redB,
                            in_=rs_outB.rearrange("(n p) c -> p n c", p=P))

        # residual input (x[b, my tokens] + bm, folded on host); kT slot
        xres = mp.tile([P, TL // P, H], F32, tag="t_16b")
        nc.gpsimd.dma_start(out=xres,
                            in_=xres_in.ap().rearrange("(n p) c -> p n c", p=P))

        h1 = mp.tile([P, TL // P, H], F32, tag="t_xT")       # 16K
        h1T = mp.tile([P, H // P, TL], BF16, tag="t_8b")     # 8K (wk slot)
        # W2 split into two 16K halves: vv slot + one attT slot
        w2a = mp.tile([P, 8, H], BF16, tag="t_16c")
        nc.gpsimd.dma_start(
            out=w2a, in_=w2_in.ap().rearrange("(n p) f -> p n f", p=P)[:, 0:8, :])
        w2b = mp.tile([P, 8, H], BF16, tag="t_attT", bufs=2)
        nc.gpsimd.dma_start(
            out=w2b, in_=w2_in.ap().rearrange("(n p) f -> p n f", p=P)[:, 8:16, :])

        # ---- residual + LN1 (per 128-token tile) ----
        for mt in range(TL // P):
            hin = mp.tile([P, H], F32, tag="t_hin", bufs=2)
            nc.vector.tensor_add(out=hin, in0=merge_redA[:, mt, :],
                                 in1=xres[:, mt, :])
            nc.vector.tensor_add(out=hin, in0=hin, in1=merge_redB[:, mt, :])
            _ln(nc, mp, hin, h1[:, mt, :], g1_b, be1_b)

        # transpose h1 -> h1T (bf16) via PE
        for ct in range(H // P):
            for mt in range(TL // P):
                pst = pp.tile([P, P], F32, tag="ps_att", bufs=2)
                nc.tensor.transpose(pst, h1[:, mt, ct * P:(ct + 1) * P], ident)
                nc.vector.tensor_copy(out=h1T[:, ct, mt * P:(mt + 1) * P],
                                      in_=pst)

        # ---- FFN1: midT [f, tok] = relu(W1^T h1T + b1); wm slot ----
        midT = mp.tile([P, FF // P, TL], BF16, tag="t_16a")
        for ft in range(FF // P):
            psf = pp.tile([P, TL], F32, tag="ps_small")
            for kt in range(H // P):
                nc.tensor.matmul(psf, w1_s[:, kt, ft * P:(ft + 1) * P],
                                 h1T[:, kt, :],
                                 start=(kt == 0), stop=(kt == H // P - 1))
            nc.scalar.activation(out=midT[:, ft, :], in_=psf, func=AF.Relu,
                                 bias=b1_t[:, ft:ft + 1])

        # ---- FFN2 + residual + LN2 -> out ----
        out_r = out_tok.ap().rearrange("(n p) c -> p n c", p=P)
        for mt in range(TL // P):
            pso = pp.tile([P, H], F32, tag="ps_big")
            for nt in range(H // 512):
                for kt in range(FF // P):
                    w2t = w2a if kt < 8 else w2b
                    nc.tensor.matmul(
                        pso[:, nt * 512:(nt + 1) * 512],
                        midT[:, kt, mt * P:(mt + 1) * P],
                        w2t[:, kt % 8, nt * 512:(nt + 1) * 512],
                        start=(kt == 0), stop=(kt == FF // P - 1))
            h2 = mp.tile([P, H], F32, tag="t_hin", bufs=2)
            nc.vector.tensor_add(out=h2, in0=pso, in1=b2_b)
            nc.vector.tensor_add(out=h2, in0=h2, in1=h1[:, mt, :])
            o_t = mp.tile([P, H], F32, tag="t_out", bufs=2)
            _ln(nc, mp, h2, o_t, g2_b, be2_b)
            nc.sync.dma_start(out=out_r[:, mt, :], in_=o_t)

    nc.finalize()
    return nc


def _shard_inputs(x, y, Wq, bq, Wk, bk, Wv, bv, Wm, bm, W1, b1, W2, b2,
                  g1, be1, g2, be2):
    bf = ml_dtypes.bfloat16
    f32 = np.float32
    in_maps = []
    for c in range(NCORES):
        b, g = c // 2, c % 2
        cs = slice(g * HL, (g + 1) * HL)
        ts_ = slice(g * TL, (g + 1) * TL)
        m = {
            "xT": np.ascontiguousarray(x[b].T).astype(bf),
            "yT": np.ascontiguousarray(y[b].T).astype(bf),
            "xres": (x[b, ts_] + bm[None, :]).astype(f32),
            "wq": np.ascontiguousarray(Wq[:, cs]).astype(bf),
            "wk": np.ascontiguousarray(Wk[:, cs]).astype(bf),
            "wv": np.ascontiguousarray(Wv[:, cs]).astype(bf),
            "wm": np.ascontiguousarray(Wm[cs, :]).astype(bf),
            "w1": W1.astype(bf),
            "w2": W2.astype(bf),
            "bq": np.ascontiguousarray(bq[cs]).astype(f32),
            "bk": np.ascontiguousarray(bk[cs]).astype(f32),
            "bv": np.ascontiguousarray(bv[cs]).astype(bf),
            "b1": b1.astype(f32),
            "b2": b2.astype(bf),
            "g1": g1.astype(bf),
            "be1": be1.astype(bf),
            "g2": g2.astype(bf),
            "be2": be2.astype(bf),
        }
        in_maps.append(m)
    return in_maps


def kernel(**inputs):
    global _BUILT, LAST_EXEC_NS, LAST_RESULT
    inputs = {k: np.asarray(v) for k, v in inputs.items()}
    if _BUILT is None:
        _BUILT = _build()
    nc = _BUILT
    in_maps = _shard_inputs(**inputs)
    r = run_bass_kernel_spmd(nc, in_maps, core_ids=list(range(NCORES)),
                             trace=True)
    LAST_EXEC_NS = r.exec_time_ns
    LAST_RESULT = r

    out_full = np.empty((B, LQ, H), np.float32)
    attT_all = np.empty((B, NH, LK, LQ), np.float32)
    for c in range(NCORES):
        b, g = c // 2, c % 2
        attT_all[b, g * NHL:(g + 1) * NHL] = r.results[c]["att_out"]
        out_full[b, g * TL:(g + 1) * TL] = r.results[c]["out_tok"]
    att_map = attT_all.transpose(0, 1, 3, 2)
    return out_full, att_map


# revision 11
# speedup vs baseline: 1.0917x; 1.0917x over previous
"""Trainium2 Bass kernel: cross-attention transformer block (sigmoid attention).

Sharding (8 NeuronCores): data-parallel over batch B=4 (pairs of cores),
tensor-parallel degree 2 within each pair (8 of 16 heads per core for
attention; 512 of 1024 query tokens per core for merge/FFN; two pair-wise
ReduceScatters over token halves re-shard between the two, the first one
overlapped with the second half of attention).

Matmuls run in bf16 (fp32 accumulate in PSUM).  The att_map output is
produced transposed ([kk, q]) so that the att @ v matmul needs no on-chip
transpose; the host assembles the final output as a transposed view.

All large DRAM tensors use a host-swizzled layout ([p, n, ...] with the
SBUF partition index outermost) so every DMA moves per-partition-contiguous
runs at full bandwidth.

SBUF tags are shared between phase-disjoint tensors of equal slot size to
keep the static footprint under the 192 KB/partition budget.
"""

import sys

if "/opt/trn_rl_repo" not in sys.path:
    sys.path.insert(0, "/opt/trn_rl_repo")

import ml_dtypes
import numpy as np

import concourse.bass as bass
import concourse.tile as tile
from concourse import bacc, mybir
from concourse.bass_utils import run_bass_kernel_spmd
from concourse.masks import make_identity

BF16 = mybir.dt.bfloat16
F32 = mybir.dt.float32
AF = mybir.ActivationFunctionType
OP = mybir.AluOpType

B, LQ, LK, H, NH, DH, FF = 4, 1024, 2048, 1024, 16, 64, 2048
EPS = 1e-6
NCORES = 8
HL = H // 2     # local head channels (8 heads x 64)
TL = LQ // 2    # local tokens for merge/FFN
NHL = NH // 2   # local heads
P = 128
KC = H // P     # 8 contraction tiles over H
NQ = LQ // 512  # 2

LAST_EXEC_NS = None
LAST_RESULT = None
_BUILT = None


def _ln(nc, mp, hin, out_ap, g_b, be_b):
    """out = g * (hin - mean) / (std_ddof1 + eps) + be, over free dim (H)."""
    stats = mp.tile([P, 2, 6], F32, tag="t_stats", bufs=2)
    nc.vector.bn_stats(out=stats[:, 0, :], in_=hin[:, 0:512])
    nc.vector.bn_stats(out=stats[:, 1, :], in_=hin[:, 512:1024])
    mv = mp.tile([P, 2], F32, tag="t_mv", bufs=2)
    nc.vector.bn_aggr(out=mv, in_=stats)
    std = mp.tile([P, 1], F32, tag="t_std", bufs=2)
    nc.scalar.activation(out=std, in_=mv[:, 1:2], func=AF.Sqrt,
                         scale=float(H) / float(H - 1))
    nc.vector.tensor_scalar_add(out=std, in0=std, scalar1=EPS)
    rstd = mp.tile([P, 1], F32, tag="t_rstd", bufs=2)
    nc.vector.reciprocal(out=rstd, in_=std)
    nc.vector.tensor_scalar(out=out_ap, in0=hin, scalar1=mv[:, 0:1],
                            scalar2=rstd, op0=OP.subtract, op1=OP.mult)
    nc.vector.tensor_mul(out=out_ap, in0=out_ap, in1=g_b)
    nc.vector.tensor_add(out=out_ap, in0=out_ap, in1=be_b)


def _build():
    nc = bacc.Bacc("TRN2", target_bir_lowering=False, debug=False,
                   num_devices=NCORES)

    # ---- DRAM I/O (per core); big tensors in [p, n*C] swizzled layout ----
    xT_in = nc.dram_tensor("xT", [P, KC * LQ], BF16, kind="ExternalInput")
    yT_in = nc.dram_tensor("yT", [P, KC * LK], BF16, kind="ExternalInput")
    xres_in = nc.dram_tensor("xres", [P, (TL // P) * H], F32,
                             kind="ExternalInput")
    wq_in = nc.dram_tensor("wq", [P, KC * HL], BF16, kind="ExternalInput")
    wk_in = nc.dram_tensor("wk", [P, KC * HL], BF16, kind="ExternalInput")
    wv_in = nc.dram_tensor("wv", [P, KC * HL], BF16, kind="ExternalInput")
    wm_in = nc.dram_tensor("wm", [P, (HL // P) * H], BF16,
                           kind="ExternalInput")
    w1_in = nc.dram_tensor("w1", [P, KC * FF], BF16, kind="ExternalInput")
    w2_in = nc.dram_tensor("w2", [P, (FF // P) * H], BF16,
                           kind="ExternalInput")
    bq_in = nc.dram_tensor("bq", [HL], F32, kind="ExternalInput")
    bk_in = nc.dram_tensor("bk", [HL], F32, kind="ExternalInput")
    bv_in = nc.dram_tensor("bv", [HL], BF16, kind="ExternalInput")
    b1_in = nc.dram_tensor("b1", [FF], F32, kind="ExternalInput")
    b2_in = nc.dram_tensor("b2", [H], BF16, kind="ExternalInput")
    g1_in = nc.dram_tensor("g1", [H], BF16, kind="ExternalInput")
    be1_in = nc.dram_tensor("be1", [H], BF16, kind="ExternalInput")
    g2_in = nc.dram_tensor("g2", [H], BF16, kind="ExternalInput")
    be2_in = nc.dram_tensor("be2", [H], BF16, kind="ExternalInput")

    # att output swizzled: [h, c2, p, kx, q]; kk = (c2*8+kx)*128 + p
    att_out = nc.dram_tensor("att_out", [NHL, 2, P, 8, LQ], BF16,
                             kind="ExternalOutput")
    out_tok = nc.dram_tensor("out_tok", [P, (TL // P) * H], F32,
                             kind="ExternalOutput")

    with tile.TileContext(nc) as tc, \
         tc.tile_pool(name="main", bufs=1) as mp, \
         tc.tile_pool(name="psum", bufs=2, space="PSUM") as pp, \
         tc.tile_pool(name="dram", bufs=1, space="DRAM") as dp:

        # ---- inputs on one ordered HWDGE queue (sync): earliest-needed
        # first so the first matmuls start ~10us in.
        xT = mp.tile([P, KC, LQ], BF16, tag="t_xT")          # 16K -> mergeA
        nc.sync.dma_start(out=xT, in_=xT_in.ap().rearrange(
            "p (a b) -> p a b", a=KC))
        wq_s = mp.tile([P, KC, HL], BF16, tag="t_8a")        # 8K -> attedT
        nc.sync.dma_start(out=wq_s, in_=wq_in.ap().rearrange(
            "p (a b) -> p a b", a=KC))
        wk_s = mp.tile([P, KC, HL], BF16, tag="t_8b")        # 8K -> h1T
        nc.sync.dma_start(out=wk_s, in_=wk_in.ap().rearrange(
            "p (a b) -> p a b", a=KC))
        yT = mp.tile([P, KC, LK], BF16, tag="t_yT")          # 32K -> w1
        nc.sync.dma_start(out=yT, in_=yT_in.ap().rearrange(
            "p (a b) -> p a b", a=KC))
        wv_s = mp.tile([P, KC, HL], BF16, tag="t_8c")        # 8K -> merge_redA
        nc.sync.dma_start(out=wv_s, in_=wv_in.ap().rearrange(
            "p (a b) -> p a b", a=KC))
        wm_s = mp.tile([P, HL // P, H], BF16, tag="t_16a")   # 8K -> midT
        nc.sync.dma_start(out=wm_s, in_=wm_in.ap().rearrange(
            "p (a b) -> p a b", a=HL // P))

        # ---- constants (gpsimd queue) ----
        ident = mp.tile([P, P], F32, tag="ident")
        make_identity(nc, ident)
        bq_t = mp.tile([P, HL // P], F32, tag="bq_t")
        nc.gpsimd.dma_start(out=bq_t, in_=bq_in.ap().rearrange("(n p) -> p n", p=P))
        bk_t = mp.tile([P, HL // P], F32, tag="bk_t")
        nc.gpsimd.dma_start(out=bk_t, in_=bk_in.ap().rearrange("(n p) -> p n", p=P))
        b1_t = mp.tile([P, FF // P], F32, tag="b1_t")
        nc.gpsimd.dma_start(out=b1_t, in_=b1_in.ap().rearrange("(n p) -> p n", p=P))
        bv_b = mp.tile([P, HL], BF16, tag="bv_b")
        nc.gpsimd.dma_start(out=bv_b, in_=bv_in.ap().partition_broadcast(P))
        b2_b = mp.tile([P, H], BF16, tag="b2_b")
        nc.gpsimd.dma_start(out=b2_b, in_=b2_in.ap().partition_broadcast(P))
        g1_b = mp.tile([P, H], BF16, tag="g1_b")
        nc.gpsimd.dma_start(out=g1_b, in_=g1_in.ap().partition_broadcast(P))
        be1_b = mp.tile([P, H], BF16, tag="be1_b")
        nc.gpsimd.dma_start(out=be1_b, in_=be1_in.ap().partition_broadcast(P))
        g2_b = mp.tile([P, H], BF16, tag="g2_b")
        nc.gpsimd.dma_start(out=g2_b, in_=g2_in.ap().partition_broadcast(P))
        be2_b = mp.tile([P, H], BF16, tag="be2_b")
        nc.gpsimd.dma_start(out=be2_b, in_=be2_in.ap().partition_broadcast(P))

        # w1 prefetch (slot frees when yT dies after the last k-projection)
        w1_s = mp.tile([P, KC, FF], BF16, tag="t_yT")        # 32K
        nc.gpsimd.dma_start(out=w1_s, in_=w1_in.ap().rearrange(
            "p (a b) -> p a b", a=KC))

        qT = mp.tile([P, HL // P, LQ], BF16, tag="t_qT")     # 8K -> merge_redB
        kT = mp.tile([P, HL // P, LK], BF16, tag="t_16b")    # 16K -> mergeB -> xres
        vv = mp.tile([P, LK // P, HL], BF16, tag="t_16c")    # 16K -> w2a

        rg = [[0, 1], [2, 3], [4, 5], [6, 7]]

        # q projection -> qT [hd, tok]
        for mt in range(HL // P):
            for nt in range(NQ):
                ps = pp.tile([P, 512], F32, tag="ps_small")
                for kt in range(KC):
                    nc.tensor.matmul(ps, wq_s[:, kt, mt * P:(mt + 1) * P],
                                     xT[:, kt, nt * 512:(nt + 1) * 512],
                                     start=(kt == 0), stop=(kt == KC - 1))
                nc.vector.tensor_scalar_add(
                    out=qT[:, mt, nt * 512:(nt + 1) * 512], in0=ps,
                    scalar1=bq_t[:, mt:mt + 1])

        def k_proj(hc):
            for nt in range(LK // 512):
                ps = pp.tile([P, 512], F32, tag="ps_small", name=f"psk{hc}{nt}")
                for kt in range(KC):
                    nc.tensor.matmul(ps, wk_s[:, kt, hc * P:(hc + 1) * P],
                                     yT[:, kt, nt * 512:(nt + 1) * 512],
                                     start=(kt == 0), stop=(kt == KC - 1))
                nc.vector.tensor_scalar_add(
                    out=kT[:, hc, nt * 512:(nt + 1) * 512], in0=ps,
                    scalar1=bk_t[:, hc:hc + 1])

        def v_proj():
            for tt in range(LK // P):
                ps = pp.tile([P, 512], F32, tag="ps_small", name=f"psv{tt}")
                for kt in range(KC):
                    nc.tensor.matmul(ps, yT[:, kt, tt * P:(tt + 1) * P],
                                     wv_s[:, kt, :],
                                     start=(kt == 0), stop=(kt == KC - 1))
                nc.vector.tensor_add(out=vv[:, tt, :], in0=ps, in1=bv_b)

        def scores_block(h, hc, hp):
            tiles = []
            for c2 in range(2):
                attT = mp.tile([P, 8, LQ], BF16, tag="t_attT", bufs=2,
                               name=f"attT{h}_{c2}")
                tiles.append(attT)
                for kx in range(8):
                    kkt = c2 * 8 + kx
                    pss = pp.tile([P, LQ], F32, tag="ps_big",
                                  name=f"pss{h}_{kkt}")
                    for qn in range(NQ):
                        nc.tensor.matmul(
                            pss[:, qn * 512:(qn + 1) * 512],
                            kT[hp:hp + 64, hc, kkt * P:(kkt + 1) * P],
                            qT[hp:hp + 64, hc, qn * 512:(qn + 1) * 512],
                            start=True, stop=True)
                    nc.scalar.activation(out=attT[:, kx, :], in_=pss,
                                         func=AF.Sigmoid, scale=1.0 / 8.0)
                nc.sync.dma_start(out=att_out.ap()[h, c2], in_=attT)
            return tiles

        def atted_block(h, hp, tiles, psa):
            for c2 in range(2):
                attT = tiles[c2]
                for kx in range(8):
                    kkt = c2 * 8 + kx
                    for qn in range(NQ):
                        nc.tensor.matmul(
                            psa[qn][hp:hp + 64, :],
                            vv[:, kkt, h * DH:(h + 1) * DH],
                            attT[:, kx, qn * 512:(qn + 1) * 512],
                            start=(kkt == 0), stop=(kkt == LK // P - 1),
                            tile_position=(0, hp))

        attedT = mp.tile([P, HL // P, LQ], BF16, tag="t_8a")  # 8K (wq slot)

        def merge_chunk(kts, sb):
            # partial merge (head-pairs kts) over all 1024 q tokens
            for mt in range(LQ // P):
                for nt in range(H // 512):
                    psm = pp.tile([P, 512], F32, tag="ps_small",
                                  name=f"psm{kts[0]}_{mt}_{nt}")
                    for j, kt in enumerate(kts):
                        nc.tensor.matmul(
                            psm,
                            attedT[:, kt, mt * P:(mt + 1) * P],
                            wm_s[:, kt, nt * 512:(nt + 1) * 512],
                            start=(j == 0), stop=(j == len(kts) - 1))
                    nc.vector.tensor_copy(
                        out=sb[:, mt, nt * 512:(nt + 1) * 512], in_=psm)

        # ---- attention: q -> k0 -> scores(h0) -> v -> atted(h0) -> ... ----
        for pair in range(NHL // 2):
            hc = pair
            k_proj(hc)
            psa = [pp.tile([P, 512], F32, tag="ps_att", bufs=2,
                           name=f"psa{pair}_{qn}") for qn in range(NQ)]
            for hp_i in range(2):
                h = 2 * pair + hp_i
                hp = hp_i * 64
                tiles = scores_block(h, hc, hp)
                if pair == 0 and hp_i == 0:
                    v_proj()   # first scores already queued; ACT starts early
                atted_block(h, hp, tiles, psa)
            for qn in range(NQ):
                nc.vector.tensor_copy(
                    out=attedT[:, hc, qn * 512:(qn + 1) * 512], in_=psa[qn])

            if pair == 1:
                # merge partial A (head-pairs 0-1) + RS_A overlapping the
                # second half of attention
                merge_sbA = mp.tile([P, LQ // P, H], BF16, tag="t_xT")
                merge_chunk((0, 1), merge_sbA)
                rs_inA = dp.tile([2, P, (TL // P) * H], BF16)
                rs_outA = dp.tile([P, (TL // P) * H], BF16)
                nc.gpsimd.dma_start(
                    out=rs_inA[:].rearrange("h p (n c) -> p h n c", n=TL // P),
                    in_=merge_sbA[:].rearrange("p (h n) c -> p h n c", h=2))
                nc.gpsimd.collective_compute(
                    "ReduceScatter", OP.add, replica_groups=rg,
                    ins=[rs_inA.opt()], outs=[rs_outA.opt()])
                merge_redA = mp.tile([P, TL // P, H], BF16, tag="t_8c")
                nc.gpsimd.dma_start(
                    out=merge_redA,
                    in_=rs_outA[:].rearrange("p (n c) -> p n c", n=TL // P))

        # ---- merge partial B (head-pairs 2-3) + RS_B ----
        merge_sbB = mp.tile([P, LQ // P, H], BF16, tag="t_16b")
        merge_chunk((2, 3), merge_sbB)
        rs_inB = dp.tile([2, P, (TL // P) * H], BF16)
        rs_outB = dp.tile([P, (TL // P) * H], BF16)
        nc.gpsimd.dma_start(
            out=rs_inB[:].rearrange("h p (n c) -> p h n c", n=TL // P),
            in_=merge_sbB[:].rearrange("p (h n) c -> p h n c", h=2))
        nc.gpsimd.collective_compute(
            "ReduceScatter", OP.add, replica_groups=rg,
            ins=[rs_inB.opt()], outs=[rs_outB.opt()])
        merge_redB = mp.tile([P, TL // P, H], BF16, tag="t_qT")
        nc.gpsimd.dma_start(
            out=merge_redB,
            in_=rs_outB[:].rearrange("p (n c) -> p n c", n=TL // P))

        # residual input (x[b, my tokens] + bm, folded on host); kT slot
        xres = mp.tile([P, TL // P, H], F32, tag="t_16b")
        nc.gpsimd.dma_start(out=xres, in_=xres_in.ap().rearrange(
            "p (n c) -> p n c", n=TL // P))

        h1 = mp.tile([P, TL // P, H], F32, tag="t_xT")       # 16K
        h1T = mp.tile([P, H // P, TL], BF16, tag="t_8b")     # 8K (wk slot)
        # W2 split into two 16K halves: vv slot + one attT slot
        w2a = mp.tile([P, 8, H], BF16, tag="t_16c")
        nc.gpsimd.dma_start(out=w2a, in_=w2_in.ap().rearrange(
            "p (a b) -> p a b", a=FF // P)[:, 0:8, :])
        w2b = mp.tile([P, 8, H], BF16, tag="t_attT", bufs=2)
        nc.gpsimd.dma_start(out=w2b, in_=w2_in.ap().rearrange(
            "p (a b) -> p a b", a=FF // P)[:, 8:16, :])

        # ---- residual + LN1 (per 128-token tile) ----
        for mt in range(TL // P):
            hin = mp.tile([P, H], F32, tag="t_hin", bufs=2)
            nc.vector.tensor_add(out=hin, in0=merge_redA[:, mt, :],
                                 in1=xres[:, mt, :])
            nc.vector.tensor_add(out=hin, in0=hin, in1=merge_redB[:, mt, :])
            _ln(nc, mp, hin, h1[:, mt, :], g1_b, be1_b)

        # transpose h1 -> h1T (bf16) via PE
        for ct in range(H // P):
            for mt in range(TL // P):
                pst = pp.tile([P, P], F32, tag="ps_att", bufs=2)
                nc.tensor.transpose(pst, h1[:, mt, ct * P:(ct + 1) * P], ident)
                nc.vector.tensor_copy(out=h1T[:, ct, mt * P:(mt + 1) * P],
                                      in_=pst)

        # ---- FFN1: midT [f, tok] = relu(W1^T h1T + b1); wm slot ----
        midT = mp.tile([P, FF // P, TL], BF16, tag="t_16a")
        for ft in range(FF // P):
            psf = pp.tile([P, TL], F32, tag="ps_small")
            for kt in range(H // P):
                nc.tensor.matmul(psf, w1_s[:, kt, ft * P:(ft + 1) * P],
                                 h1T[:, kt, :],
                                 start=(kt == 0), stop=(kt == H // P - 1))
            nc.scalar.activation(out=midT[:, ft, :], in_=psf, func=AF.Relu,
                                 bias=b1_t[:, ft:ft + 1])

        # ---- FFN2 + residual + LN2 -> out ----
        out_r = out_tok.ap().rearrange("p (n c) -> p n c", n=TL // P)
        for mt in range(TL // P):
            pso = pp.tile([P, H], F32, tag="ps_big")
            for nt in range(H // 512):
                for kt in range(FF // P):
                    w2t = w2a if kt < 8 else w2b
                    nc.tensor.matmul(
                        pso[:, nt * 512:(nt + 1) * 512],
                        midT[:, kt, mt * P:(mt + 1) * P],
                        w2t[:, kt % 8, nt * 512:(nt + 1) * 512],
                        start=(kt == 0), stop=(kt == FF // P - 1))
            h2 = mp.tile([P, H], F32, tag="t_hin", bufs=2)
            nc.vector.tensor_add(out=h2, in0=pso, in1=b2_b)
            nc.vector.tensor_add(out=h2, in0=h2, in1=h1[:, mt, :])
            o_t = mp.tile([P, H], F32, tag="t_out", bufs=2)
            _ln(nc, mp, h2, o_t, g2_b, be2_b)
            nc.sync.dma_start(out=out_r[:, mt, :], in_=o_t)

    nc.finalize()
    return nc


def _sw(a):
    """[n*128, C] row-major -> [128, n*C]: partition index outermost."""
    R, C = a.shape
    n = R // P
    return np.ascontiguousarray(
        a.reshape(n, P, C).transpose(1, 0, 2).reshape(P, n * C))


def _shard_inputs(x, y, Wq, bq, Wk, bk, Wv, bv, Wm, bm, W1, b1, W2, b2,
                  g1, be1, g2, be2):
    bf = ml_dtypes.bfloat16
    f32 = np.float32
    in_maps = []
    for c in range(NCORES):
        b, g = c // 2, c % 2
        cs = slice(g * HL, (g + 1) * HL)
        ts_ = slice(g * TL, (g + 1) * TL)
        m = {
            "xT": _sw(np.ascontiguousarray(x[b].T).astype(bf)),
            "yT": _sw(np.ascontiguousarray(y[b].T).astype(bf)),
            "xres": _sw((x[b, ts_] + bm[None, :]).astype(f32)),
            "wq": _sw(np.ascontiguousarray(Wq[:, cs]).astype(bf)),
            "wk": _sw(np.ascontiguousarray(Wk[:, cs]).astype(bf)),
            "wv": _sw(np.ascontiguousarray(Wv[:, cs]).astype(bf)),
            "wm": _sw(np.ascontiguousarray(Wm[cs, :]).astype(bf)),
            "w1": _sw(W1.astype(bf)),
            "w2": _sw(W2.astype(bf)),
            "bq": np.ascontiguousarray(bq[cs]).astype(f32),
            "bk": np.ascontiguousarray(bk[cs]).astype(f32),
            "bv": np.ascontiguousarray(bv[cs]).astype(bf),
            "b1": b1.astype(f32),
            "b2": b2.astype(bf),
            "g1": g1.astype(bf),
            "be1": be1.astype(bf),
            "g2": g2.astype(bf),
            "be2": be2.astype(bf),
        }
        in_maps.append(m)
    return in_maps


def kernel(**inputs):
    global _BUILT, LAST_EXEC_NS, LAST_RESULT
    inputs = {k: np.asarray(v) for k, v in inputs.items()}
    if _BUILT is None:
        _BUILT = _build()
    nc = _BUILT
    in_maps = _shard_inputs(**inputs)
    r = run_bass_kernel_spmd(nc, in_maps, core_ids=list(range(NCORES)),
                             trace=True)
    LAST_EXEC_NS = r.exec_time_ns
    LAST_RESULT = r

    out_full = np.empty((B, LQ, H), np.float32)
    attT_all = np.empty((B, NH, LK, LQ), np.float32)
    for c in range(NCORES):
        b, g = c // 2, c % 2
        # att_out [h, c2, p, kx, q] -> [h, (c2 kx p) = kk, q]
        a = r.results[c]["att_out"]
        attT_all[b, g * NHL:(g + 1) * NHL] = \
            a.transpose(0, 1, 3, 2, 4).reshape(NHL, LK, LQ)
        # out_tok [p, mt*H + c] -> [tok, c]
        o = r.results[c]["out_tok"]
        out_full[b, g * TL:(g + 1) * TL] = \
            o.reshape(P, TL // P, H).transpose(1, 0, 2).reshape(TL, H)
    att_map = attT_all.transpose(0, 1, 3, 2)
    return out_full, att_map


# revision 19
# speedup vs baseline: 1.1500x; 1.0534x over previous
"""Trainium2 Bass kernel: cross-attention transformer block (sigmoid attention).

Sharding (8 NeuronCores): data-parallel over batch B=4 (pairs of cores),
tensor-parallel degree 2 within each pair (8 of 16 heads per core for
attention; 512 of 1024 query tokens per core for merge/FFN; two pair-wise
ReduceScatters over token halves re-shard between the two, the first one
overlapped with the second half of attention).

Matmuls run in bf16 (fp32 accumulate in PSUM).  The att_map output is
produced transposed ([kk, q]) so that the att @ v matmul needs no on-chip
transpose; the host assembles the final output as a transposed view.

All large DRAM tensors use a host-swizzled layout ([p, n, ...] with the
SBUF partition index outermost) so every DMA moves per-partition-contiguous
runs at full bandwidth.

SBUF tags are shared between phase-disjoint tensors of equal slot size to
keep the static footprint under the 192 KB/partition budget.
"""

import sys

if "/opt/trn_rl_repo" not in sys.path:
    sys.path.insert(0, "/opt/trn_rl_repo")

import ml_dtypes
import numpy as np

import concourse.bass as bass
import concourse.tile as tile
from concourse import bacc, mybir
from concourse.bass_utils import run_bass_kernel_spmd
from concourse.masks import make_identity

BF16 = mybir.dt.bfloat16
F32 = mybir.dt.float32
AF = mybir.ActivationFunctionType
OP = mybir.AluOpType

B, LQ, LK, H, NH, DH, FF = 4, 1024, 2048, 1024, 16, 64, 2048
EPS = 1e-6
NCORES = 8
HL = H // 2     # local head channels (8 heads x 64)
TL = LQ // 2    # local tokens for merge/FFN
NHL = NH // 2   # local heads
P = 128
KC = H // P     # 8 contraction tiles over H
NQ = LQ // 512  # 2

LAST_EXEC_NS = None
LAST_RESULT = None
_BUILT = None


def _ln(nc, mp, hin, out_ap, g_b, be_b):
    """out = g * (hin - mean) / (std_ddof1 + eps) + be, over free dim (H)."""
    stats = mp.tile([P, 2, 6], F32, tag="t_stats", bufs=2)
    nc.vector.bn_stats(out=stats[:, 0, :], in_=hin[:, 0:512])
    nc.vector.bn_stats(out=stats[:, 1, :], in_=hin[:, 512:1024])
    mv = mp.tile([P, 2], F32, tag="t_mv", bufs=2)
    nc.vector.bn_aggr(out=mv, in_=stats)
    std = mp.tile([P, 1], F32, tag="t_std", bufs=2)
    nc.scalar.activation(out=std, in_=mv[:, 1:2], func=AF.Sqrt,
                         scale=float(H) / float(H - 1))
    nc.vector.tensor_scalar_add(out=std, in0=std, scalar1=EPS)
    rstd = mp.tile([P, 1], F32, tag="t_rstd", bufs=2)
    nc.vector.reciprocal(out=rstd, in_=std)
    nc.vector.tensor_scalar(out=out_ap, in0=hin, scalar1=mv[:, 0:1],
                            scalar2=rstd, op0=OP.subtract, op1=OP.mult)
    nc.vector.tensor_mul(out=out_ap, in0=out_ap, in1=g_b)
    nc.vector.tensor_add(out=out_ap, in0=out_ap, in1=be_b)


def _build():
    nc = bacc.Bacc("TRN2", target_bir_lowering=False, debug=False,
                   num_devices=NCORES)

    # ---- DRAM I/O (per core); big tensors in [p, n*C] swizzled layout ----
    xT_in = nc.dram_tensor("xT", [P, KC * LQ], BF16, kind="ExternalInput")
    yT_in = nc.dram_tensor("yT", [P, KC * LK], BF16, kind="ExternalInput")
    xres_in = nc.dram_tensor("xres", [P, (TL // P) * H], F32,
                             kind="ExternalInput")
    wq_in = nc.dram_tensor("wq", [P, KC * HL], BF16, kind="ExternalInput")
    wk_in = nc.dram_tensor("wk", [P, KC * HL], BF16, kind="ExternalInput")
    wv_in = nc.dram_tensor("wv", [P, KC * HL], BF16, kind="ExternalInput")
    wm_in = nc.dram_tensor("wm", [P, (HL // P) * H], BF16,
                           kind="ExternalInput")
    w1_in = nc.dram_tensor("w1", [P, KC * FF], BF16, kind="ExternalInput")
    w2_in = nc.dram_tensor("w2", [P, (FF // P) * H], BF16,
                           kind="ExternalInput")
    bq_in = nc.dram_tensor("bq", [HL], F32, kind="ExternalInput")
    bk_in = nc.dram_tensor("bk", [HL], F32, kind="ExternalInput")
    bv_in = nc.dram_tensor("bv", [HL], BF16, kind="ExternalInput")
    b1_in = nc.dram_tensor("b1", [FF], F32, kind="ExternalInput")
    b2_in = nc.dram_tensor("b2", [H], BF16, kind="ExternalInput")
    g1_in = nc.dram_tensor("g1", [H], BF16, kind="ExternalInput")
    be1_in = nc.dram_tensor("be1", [H], BF16, kind="ExternalInput")
    g2_in = nc.dram_tensor("g2", [H], BF16, kind="ExternalInput")
    be2_in = nc.dram_tensor("be2", [H], BF16, kind="ExternalInput")

    # att output swizzled: [h, c2, p, kx, q]; kk = (c2*8+kx)*128 + p
    att_out = nc.dram_tensor("att_out", [NHL, 2, P, 8, LQ], BF16,
                             kind="ExternalOutput")
    out_tok = nc.dram_tensor("out_tok", [P, (TL // P) * H], F32,
                             kind="ExternalOutput")

    with tile.TileContext(nc) as tc, \
         tc.tile_pool(name="main", bufs=1) as mp, \
         tc.tile_pool(name="psum", bufs=2, space="PSUM") as pp, \
         tc.tile_pool(name="dram", bufs=1, space="DRAM") as dp:

        # ---- inputs on one ordered HWDGE queue (sync): earliest-needed
        # first so the first matmuls start ~10us in.
        xT = mp.tile([P, KC, LQ], BF16, tag="t_xT")          # 16K -> mergeA
        nc.sync.dma_start(out=xT, in_=xT_in.ap().rearrange(
            "p (a b) -> p a b", a=KC))
        wq_s = mp.tile([P, KC, HL], BF16, tag="t_8a")        # 8K -> attedT
        nc.sync.dma_start(out=wq_s, in_=wq_in.ap().rearrange(
            "p (a b) -> p a b", a=KC))
        wk_s = mp.tile([P, KC, HL], BF16, tag="t_8b")        # 8K -> h1T
        nc.sync.dma_start(out=wk_s, in_=wk_in.ap().rearrange(
            "p (a b) -> p a b", a=KC))
        yT = mp.tile([P, KC, LK], BF16, tag="t_yT")          # 32K -> w1
        nc.sync.dma_start(out=yT, in_=yT_in.ap().rearrange(
            "p (a b) -> p a b", a=KC))
        wv_s = mp.tile([P, KC, HL], BF16, tag="t_8c")        # 8K -> merge_redA
        nc.sync.dma_start(out=wv_s, in_=wv_in.ap().rearrange(
            "p (a b) -> p a b", a=KC))
        wm_s = mp.tile([P, HL // P, H], BF16, tag="t_16a")   # 8K -> midT
        nc.sync.dma_start(out=wm_s, in_=wm_in.ap().rearrange(
            "p (a b) -> p a b", a=HL // P))

        # ---- constants (gpsimd queue) ----
        ident = mp.tile([P, P], F32, tag="ident")
        make_identity(nc, ident)
        bq_t = mp.tile([P, HL // P], F32, tag="bq_t")
        nc.gpsimd.dma_start(out=bq_t, in_=bq_in.ap().rearrange("(n p) -> p n", p=P))
        bk_t = mp.tile([P, HL // P], F32, tag="bk_t")
        nc.gpsimd.dma_start(out=bk_t, in_=bk_in.ap().rearrange("(n p) -> p n", p=P))
        b1_t = mp.tile([P, FF // P], F32, tag="b1_t")
        nc.gpsimd.dma_start(out=b1_t, in_=b1_in.ap().rearrange("(n p) -> p n", p=P))
        bv_b = mp.tile([P, HL], BF16, tag="bc1")
        nc.gpsimd.dma_start(out=bv_b, in_=bv_in.ap().partition_broadcast(P))
        g1_b = mp.tile([P, H], BF16, tag="bc2")
        nc.gpsimd.dma_start(out=g1_b, in_=g1_in.ap().partition_broadcast(P))
        be1_b = mp.tile([P, H], BF16, tag="bc3")
        nc.gpsimd.dma_start(out=be1_b, in_=be1_in.ap().partition_broadcast(P))

        # w1 prefetch (slot frees when yT dies after the last k-projection)
        w1_s = mp.tile([P, KC, FF], BF16, tag="t_yT")        # 32K
        nc.gpsimd.dma_start(out=w1_s, in_=w1_in.ap().rearrange(
            "p (a b) -> p a b", a=KC))

        qT = mp.tile([P, HL // P, LQ], BF16, tag="t_qT")     # 8K -> merge_redB
        kT = mp.tile([P, HL // P, LK], BF16, tag="t_16b")    # 16K -> mergeB -> xres
        vv = mp.tile([P, LK // P, HL], BF16, tag="t_16c")    # 16K -> w2a

        rg = [[0, 1], [2, 3], [4, 5], [6, 7]]

        # q projection -> qT [hd, tok]
        for mt in range(HL // P):
            for nt in range(NQ):
                ps = pp.tile([P, 512], F32, tag="ps_small")
                for kt in range(KC):
                    nc.tensor.matmul(ps, wq_s[:, kt, mt * P:(mt + 1) * P],
                                     xT[:, kt, nt * 512:(nt + 1) * 512],
                                     start=(kt == 0), stop=(kt == KC - 1))
                nc.vector.tensor_scalar_add(
                    out=qT[:, mt, nt * 512:(nt + 1) * 512], in0=ps,
                    scalar1=bq_t[:, mt:mt + 1])

        def k_chain(hc, nt):
            ps = pp.tile([P, 512], F32, tag="ps_small", name=f"psk{hc}{nt}")
            for kt in range(KC):
                nc.tensor.matmul(ps, wk_s[:, kt, hc * P:(hc + 1) * P],
                                 yT[:, kt, nt * 512:(nt + 1) * 512],
                                 start=(kt == 0), stop=(kt == KC - 1))
            nc.vector.tensor_scalar_add(
                out=kT[:, hc, nt * 512:(nt + 1) * 512], in0=ps,
                scalar1=bk_t[:, hc:hc + 1])

        def k_proj(hc):
            for nt in range(LK // 512):
                k_chain(hc, nt)

        def v_chain(tt):
            ps = pp.tile([P, 512], F32, tag="ps_small", name=f"psv{tt}")
            for kt in range(KC):
                nc.tensor.matmul(ps, yT[:, kt, tt * P:(tt + 1) * P],
                                 wv_s[:, kt, :],
                                 start=(kt == 0), stop=(kt == KC - 1))
            nc.vector.tensor_add(out=vv[:, tt, :], in0=ps, in1=bv_b)

        attedT = mp.tile([P, HL // P, LQ], BF16, tag="t_8a")  # 8K (wq slot)

        def merge_chunk(kts, sb):
            # partial merge (head-pairs kts) over all 1024 q tokens
            for mt in range(LQ // P):
                for nt in range(H // 512):
                    psm = pp.tile([P, 512], F32, tag="ps_small",
                                  name=f"psm{kts[0]}_{mt}_{nt}")
                    for j, kt in enumerate(kts):
                        nc.tensor.matmul(
                            psm,
                            attedT[:, kt, mt * P:(mt + 1) * P],
                            wm_s[:, kt, nt * 512:(nt + 1) * 512],
                            start=(j == 0), stop=(j == len(kts) - 1))
                    nc.vector.tensor_copy(
                        out=sb[:, mt, nt * 512:(nt + 1) * 512], in_=psm)

        # ---- attention: head PAIRS processed jointly, the two heads' 64-wide
        # matmuls packed onto disjoint PE row/col groups (concurrent).
        # v-projection chains interleave into pair 0's scores stream and the
        # next pair's k-projection into the atted stream to keep ACT fed.
        for pair in range(NHL // 2):
            hc = pair
            if pair == 0:
                k_proj(0)
            for c2 in range(2):
                tiles = []
                for hp_i in range(2):
                    attT = mp.tile([P, 8, LQ], BF16, tag="t_attT", bufs=3,
                                   name=f"attT{2 * pair + hp_i}_{c2}")
                    tiles.append(attT)
                # scores + sigmoid, both heads per kk tile
                for kx in range(8):
                    kkt = c2 * 8 + kx
                    pss = []
                    for hp_i in range(2):
                        hp = hp_i * 64
                        ptile = pp.tile([P, LQ], F32, tag="ps_big",
                                        name=f"pss{pair}_{kkt}_{hp_i}")
                        pss.append(ptile)
                        for qn in range(NQ):
                            nc.tensor.matmul(
                                ptile[:, qn * 512:(qn + 1) * 512],
                                kT[hp:hp + 64, hc, kkt * P:(kkt + 1) * P],
                                qT[hp:hp + 64, hc, qn * 512:(qn + 1) * 512],
                                start=True, stop=True)
                    for hp_i in range(2):
                        nc.scalar.activation(out=tiles[hp_i][:, kx, :],
                                             in_=pss[hp_i], func=AF.Sigmoid,
                                             scale=1.0 / 8.0)
                    if pair == 0:
                        v_chain(kkt)   # fill ACT-bound bubbles with v-proj
                for hp_i in range(2):
                    nc.sync.dma_start(out=att_out.ap()[2 * pair + hp_i, c2],
                                      in_=tiles[hp_i])
                # atted: per parity a self-contained 8-matmul chain for this
                # kk half (own psum start/stop; SBUF accumulate across halves)
                for hp_i in range(2):
                    hp = hp_i * 64
                    h = 2 * pair + hp_i
                    pch = [pp.tile([P, 512], F32, tag="ps_att", bufs=2,
                                   name=f"pat{pair}{c2}{hp_i}{qn}")
                           for qn in range(NQ)]
                    for kx in range(8):
                        kkt = c2 * 8 + kx
                        for qn in range(NQ):
                            nc.tensor.matmul(
                                pch[qn][hp:hp + 64, :],
                                vv[:, kkt, h * DH:(h + 1) * DH],
                                tiles[hp_i][:, kx, qn * 512:(qn + 1) * 512],
                                start=(kx == 0), stop=(kx == 7),
                                tile_position=(0, hp))
                    for qn in range(NQ):
                        dst = attedT[hp:hp + 64, hc, qn * 512:(qn + 1) * 512]
                        if c2 == 0:
                            nc.vector.tensor_copy(out=dst,
                                                  in_=pch[qn][hp:hp + 64, :])
                        else:
                            nc.vector.tensor_add(out=dst,
                                                 in0=pch[qn][hp:hp + 64, :],
                                                 in1=dst)
                    if c2 == 1 and pair < NHL // 2 - 1:
                        k_chain(pair + 1, 2 * hp_i)
                        k_chain(pair + 1, 2 * hp_i + 1)

            if pair == 1:
                # merge partial A (head-pairs 0-1) + RS_A overlapping the
                # second half of attention
                merge_sbA = mp.tile([P, LQ // P, H], BF16, tag="t_xT")
                merge_chunk((0, 1), merge_sbA)
                rs_inA = dp.tile([2, P, (TL // P) * H], BF16)
                rs_outA = dp.tile([P, (TL // P) * H], BF16)
                nc.gpsimd.dma_start(
                    out=rs_inA[:].rearrange("h p (n c) -> p h n c", n=TL // P),
                    in_=merge_sbA[:].rearrange("p (h n) c -> p h n c", h=2))
                nc.gpsimd.collective_compute(
                    "ReduceScatter", OP.add, replica_groups=rg,
                    ins=[rs_inA.opt()], outs=[rs_outA.opt()])
                merge_redA = mp.tile([P, TL // P, H], BF16, tag="t_8c")
                nc.gpsimd.dma_start(
                    out=merge_redA,
                    in_=rs_outA[:].rearrange("p (n c) -> p n c", n=TL // P))

        # ---- merge partial B (head-pairs 2-3) + RS_B ----
        merge_sbB = mp.tile([P, LQ // P, H], BF16, tag="t_16b")
        merge_chunk((2, 3), merge_sbB)
        rs_inB = dp.tile([2, P, (TL // P) * H], BF16)
        rs_outB = dp.tile([P, (TL // P) * H], BF16)
        nc.gpsimd.dma_start(
            out=rs_inB[:].rearrange("h p (n c) -> p h n c", n=TL // P),
            in_=merge_sbB[:].rearrange("p (h n) c -> p h n c", h=2))
        nc.gpsimd.collective_compute(
            "ReduceScatter", OP.add, replica_groups=rg,
            ins=[rs_inB.opt()], outs=[rs_outB.opt()])
        merge_redB = mp.tile([P, TL // P, H], BF16, tag="t_qT")
        nc.gpsimd.dma_start(
            out=merge_redB,
            in_=rs_outB[:].rearrange("p (n c) -> p n c", n=TL // P))

        # residual input (x[b, my tokens] + bm, folded on host); kT slot
        xres = mp.tile([P, TL // P, H], F32, tag="t_16b")
        nc.gpsimd.dma_start(out=xres, in_=xres_in.ap().rearrange(
            "p (n c) -> p n c", n=TL // P))

        h1 = mp.tile([P, TL // P, H], F32, tag="t_xT")       # 16K
        # pre-add xres + redA while RS_B is in flight
        for mt in range(TL // P):
            nc.vector.tensor_add(out=h1[:, mt, :], in0=merge_redA[:, mt, :],
                                 in1=xres[:, mt, :])
        h1T = mp.tile([P, H // P, TL], BF16, tag="t_8b")     # 8K (wk slot)
        # W2 split into two 16K halves: vv slot + one attT slot
        w2a = mp.tile([P, 8, H], BF16, tag="t_16c")
        nc.gpsimd.dma_start(out=w2a, in_=w2_in.ap().rearrange(
            "p (a b) -> p a b", a=FF // P)[:, 0:8, :])
        w2b = mp.tile([P, 8, H], BF16, tag="t_attT", bufs=3)
        nc.gpsimd.dma_start(out=w2b, in_=w2_in.ap().rearrange(
            "p (a b) -> p a b", a=FF // P)[:, 8:16, :])

        # late broadcast constants (slots shared with bv/g1/be1)
        b2_b = mp.tile([P, H], BF16, tag="bc2")
        nc.gpsimd.dma_start(out=b2_b, in_=b2_in.ap().partition_broadcast(P))
        g2_b = mp.tile([P, H], BF16, tag="bc3")
        nc.gpsimd.dma_start(out=g2_b, in_=g2_in.ap().partition_broadcast(P))
        be2_b = mp.tile([P, H], BF16, tag="bc1")
        nc.gpsimd.dma_start(out=be2_b, in_=be2_in.ap().partition_broadcast(P))

        # ---- residual + LN1 + transpose (per 128-token tile) ----
        for mt in range(TL // P):
            hin = mp.tile([P, H], F32, tag="t_hin", bufs=1)
            nc.vector.tensor_add(out=hin, in0=h1[:, mt, :],
                                 in1=merge_redB[:, mt, :])
            _ln(nc, mp, hin, h1[:, mt, :], g1_b, be1_b)
            for ct in range(H // P):
                pst = pp.tile([P, P], F32, tag="ps_att", bufs=2)
                nc.tensor.transpose(pst, h1[:, mt, ct * P:(ct + 1) * P], ident)
                nc.vector.tensor_copy(out=h1T[:, ct, mt * P:(mt + 1) * P],
                                      in_=pst)

        # ---- FFN1: midT [f, tok] = relu(W1^T h1T + b1); wm slot ----
        midT = mp.tile([P, FF // P, TL], BF16, tag="t_16a")
        for ft in range(FF // P):
            psf = pp.tile([P, TL], F32, tag="ps_small")
            for kt in range(H // P):
                nc.tensor.matmul(psf, w1_s[:, kt, ft * P:(ft + 1) * P],
                                 h1T[:, kt, :],
                                 start=(kt == 0), stop=(kt == H // P - 1))
            nc.scalar.activation(out=midT[:, ft, :], in_=psf, func=AF.Relu,
                                 bias=b1_t[:, ft:ft + 1])

        # ---- FFN2 + residual + LN2 -> out ----
        out_r = out_tok.ap().rearrange("p (n c) -> p n c", n=TL // P)
        for mt in range(TL // P):
            pso = pp.tile([P, H], F32, tag="ps_big")
            for nt in range(H // 512):
                for kt in range(FF // P):
                    w2t = w2a if kt < 8 else w2b
                    nc.tensor.matmul(
                        pso[:, nt * 512:(nt + 1) * 512],
                        midT[:, kt, mt * P:(mt + 1) * P],
                        w2t[:, kt % 8, nt * 512:(nt + 1) * 512],
                        start=(kt == 0), stop=(kt == FF // P - 1))
            h2 = mp.tile([P, H], F32, tag="t_hin", bufs=1)
            nc.vector.tensor_add(out=h2, in0=pso, in1=b2_b)
            nc.vector.tensor_add(out=h2, in0=h2, in1=h1[:, mt, :])
            o_t = mp.tile([P, H], F32, tag="t_out", bufs=1)
            _ln(nc, mp, h2, o_t, g2_b, be2_b)
            nc.sync.dma_start(out=out_r[:, mt, :], in_=o_t)

    nc.finalize()
    return nc


def _sw(a):
    """[n*128, C] row-major -> [128, n*C]: partition index outermost."""
    R, C = a.shape
    n = R // P
    return np.ascontiguousarray(
        a.reshape(n, P, C).transpose(1, 0, 2).reshape(P, n * C))


def _shard_inputs(x, y, Wq, bq, Wk, bk, Wv, bv, Wm, bm, W1, b1, W2, b2,
                  g1, be1, g2, be2):
    bf = ml_dtypes.bfloat16
    f32 = np.float32
    in_maps = []
    for c in range(NCORES):
        b, g = c // 2, c % 2
        cs = slice(g * HL, (g + 1) * HL)
        ts_ = slice(g * TL, (g + 1) * TL)
        m = {
            "xT": _sw(np.ascontiguousarray(x[b].T).astype(bf)),
            "yT": _sw(np.ascontiguousarray(y[b].T).astype(bf)),
            "xres": _sw((x[b, ts_] + bm[None, :]).astype(f32)),
            "wq": _sw(np.ascontiguousarray(Wq[:, cs]).astype(bf)),
            "wk": _sw(np.ascontiguousarray(Wk[:, cs]).astype(bf)),
            "wv": _sw(np.ascontiguousarray(Wv[:, cs]).astype(bf)),
            "wm": _sw(np.ascontiguousarray(Wm[cs, :]).astype(bf)),
            "w1": _sw(W1.astype(bf)),
            "w2": _sw(W2.astype(bf)),
            "bq": np.ascontiguousarray(bq[cs]).astype(f32),
            "bk": np.ascontiguousarray(bk[cs]).astype(f32),
            "bv": np.ascontiguousarray(bv[cs]).astype(bf),
            "b1": b1.astype(f32),
            "b2": b2.astype(bf),
            "g1": g1.astype(bf),
            "be1": be1.astype(bf),
            "g2": g2.astype(bf),
            "be2": be2.astype(bf),
        }
        in_maps.append(m)
    return in_maps


def kernel(**inputs):
    global _BUILT, LAST_EXEC_NS, LAST_RESULT
    inputs = {k: np.asarray(v) for k, v in inputs.items()}
    if _BUILT is None:
        _BUILT = _build()
    nc = _BUILT
    in_maps = _shard_inputs(**inputs)
    r = run_bass_kernel_spmd(nc, in_maps, core_ids=list(range(NCORES)),
                             trace=True)
    LAST_EXEC_NS = r.exec_time_ns
    LAST_RESULT = r

    out_full = np.empty((B, LQ, H), np.float32)
    attT_all = np.empty((B, NH, LK, LQ), np.float32)
    for c in range(NCORES):
        b, g = c // 2, c % 2
        # att_out [h, c2, p, kx, q] -> [h, (c2 kx p) = kk, q]
        a = r.results[c]["att_out"]
        attT_all[b, g * NHL:(g + 1) * NHL] = \
            a.transpose(0, 1, 3, 2, 4).reshape(NHL, LK, LQ)
        # out_tok [p, mt*H + c] -> [tok, c]
        o = r.results[c]["out_tok"]
        out_full[b, g * TL:(g + 1) * TL] = \
            o.reshape(P, TL // P, H).transpose(1, 0, 2).reshape(TL, H)
    att_map = attT_all.transpose(0, 1, 3, 2)
    return out_full, att_map
